# revision 6
# baseline (speedup 1.0000x reference)
"""RNN forward kernel for Trainium2, data-parallel over batch on 8 NeuronCores.

Computation (per batch row b):
    xp[t] = W_ih @ x[b,t] + b_ih + b_hh          (input projection, d=200 -> h=64)
    h_t   = tanh(xp[t] + W_hh @ h_{t-1})         (recurrence over T=128)
    out_b = sigmoid(W_out @ h_T + b_out)

Per-core plan (512 batch rows):
  - x streamed from HBM in (b-block, t-chunk) units, cast fp32->fp16 during the
    SWDGE DMA, staged in two layouts (d 0:128 and d 128:200 padded to 80) so the
    xbar DMA-transpose can run on large 2D-contiguous inputs.
  - xbar transpose puts d on partitions: xT[d, t, b] tiles feed the PE.
  - Projection matmuls in fp16 (PSUM accumulates fp32), recurrence matmul in
    fp32, tanh(+bias) on ScalarE straight out of PSUM.
  - Final: one matmul against [W_out.T; b_out] with a ones-row appended to h,
    sigmoid on ScalarE, store.
"""

import os

os.environ.setdefault("JAX_PLATFORMS", "cpu")

import numpy as np

B, T, D, H = 4096, 128, 200, 64
N_CORES = 8
BS = B // N_CORES  # 512 rows per core
DLO = 128          # first d-chunk (partitions of xT_lo)
DHI = D - DLO      # 72
# The hi d-chunk is padded to a full 128-column period: the HW xbar transpose
# only produces the plain 2D transpose when the source fold period equals the
# 128-column xbar tile width (verified on HW; 80 scrambles).
DHIP = 128
TC_LOAD = 16       # t granularity of load + transpose
TC_PS = 8          # t granularity of one PSUM accumulation chunk


def build_program(bs=BS, t_len=T, tc_load=TC_LOAD, tc_ps=TC_PS):
    import concourse.bacc as bacc
    import concourse.mybir as mybir
    import concourse.tile as tile
    from concourse._compat import axon_active

    f32, f16 = mybir.dt.float32, mybir.dt.float16
    Tanh = mybir.ActivationFunctionType.Tanh
    Sigmoid = mybir.ActivationFunctionType.Sigmoid

    nblk = bs // 128
    nld = t_len // tc_load

    nc = bacc.Bacc("TRN2", target_bir_lowering=False, debug=not axon_active())

    x_d = nc.dram_tensor("x", [bs, t_len, D], f32, kind="ExternalInput")
    wih_d = nc.dram_tensor("W_ih", [H, D], f32, kind="ExternalInput")
    whh_d = nc.dram_tensor("W_hh", [H, H], f32, kind="ExternalInput")
    bih_d = nc.dram_tensor("b_ih", [H], f32, kind="ExternalInput")
    bhh_d = nc.dram_tensor("b_hh", [H], f32, kind="ExternalInput")
    wout_d = nc.dram_tensor("W_out", [1, H], f32, kind="ExternalInput")
    bout_d = nc.dram_tensor("b_out", [1], f32, kind="ExternalInput")
    out_d = nc.dram_tensor("out", [bs, 1], f32, kind="ExternalOutput")

    with tile.TileContext(nc) as tc:
        with (
            tc.tile_pool(name="const", bufs=1) as cpool,
            tc.tile_pool(name="stage", bufs=3) as spool,
            tc.tile_pool(name="xt", bufs=8) as xtpool,
            tc.tile_pool(name="state", bufs=1) as hpool,
            tc.tile_pool(name="psum", bufs=1, space="PSUM") as pspool,
        ):
            # ---- weight prep (tiny, once) ----
            wih_nat = cpool.tile([64, 224], f32, tag="wihnat")
            nc.gpsimd.memset(wih_nat[:], 0.0)
            nc.scalar.dma_start(out=wih_nat[:, 0:D], in_=wih_d[:, :])

            wihT_lo = cpool.tile([128, 64], f32, tag="wihTlo")
            wihT_hi = cpool.tile([96, 64], f32, tag="wihThi")
            nc.gpsimd.memset(wihT_hi[:], 0.0)
            for bi in range(7):
                dst = wihT_lo if bi < 4 else wihT_hi
                r = (bi % 4) * 32 if bi < 4 else (bi - 4) * 32
                for bj in range(2):
                    nc.vector.transpose(
                        dst[r : r + 32, bj * 32 : (bj + 1) * 32],
                        wih_nat[bj * 32 : (bj + 1) * 32, bi * 32 : (bi + 1) * 32],
                    )
            wihT_lo16 = cpool.tile([128, 64], f16, tag="wihTlo16")
            wihT_hi16 = cpool.tile([96, 64], f16, tag="wihThi16")
            nc.vector.tensor_copy(wihT_lo16[:], wihT_lo[:])
            nc.vector.tensor_copy(wihT_hi16[:], wihT_hi[:])

            wh_nat = cpool.tile([64, 64], f32, tag="whnat")
            nc.scalar.dma_start(out=wh_nat[:], in_=whh_d[:, :])
            whhT = cpool.tile([64, 64], f32, tag="whhT")
            for bi in range(2):
                for bj in range(2):
                    nc.vector.transpose(
                        whhT[bi * 32 : (bi + 1) * 32, bj * 32 : (bj + 1) * 32],
                        wh_nat[bj * 32 : (bj + 1) * 32, bi * 32 : (bi + 1) * 32],
                    )

            ba = cpool.tile([64, 1], f32, tag="ba")
            bb = cpool.tile([64, 1], f32, tag="bb")
            nc.scalar.dma_start(out=ba[:], in_=bih_d[:])
            nc.scalar.dma_start(out=bb[:], in_=bhh_d[:])
            bias = cpool.tile([64, 1], f32, tag="bias")
            nc.vector.tensor_add(bias[:], ba[:], bb[:])

            waug = cpool.tile([65, 1], f32, tag="waug")
            nc.scalar.dma_start(out=waug[0:64, :], in_=wout_d[0, :])
            nc.scalar.dma_start(out=waug[64:65, :], in_=bout_d[:])

            # ---- recurrent state (ones row at partition 64 folds b_out in) ----
            hs = []
            for k in range(nblk):
                hk = hpool.tile([65, 128], f32, tag=f"h{k}")
                nc.gpsimd.memset(hk[0:64, :], 0.0)
                nc.gpsimd.memset(hk[64:65, :], 1.0)
                hs.append(hk)

            # ---- streaming pipeline ----
            for cl in range(nld):
                tsl = slice(cl * tc_load, (cl + 1) * tc_load)
                xt_tiles = []
                for k in range(nblk):
                    bsl = slice(k * 128, (k + 1) * 128)
                    xlo_t = spool.tile([128, tc_load * DLO], f16, tag="xlo")
                    xhi_t = spool.tile([128, tc_load * DHIP], f16, tag="xhi")
                    nc.gpsimd.dma_start(
                        out=xlo_t[:].rearrange("p (t d) -> p t d", d=DLO),
                        in_=x_d[bsl, tsl, 0:DLO],
                    )
                    xhi3 = xhi_t[:].rearrange("p (t d) -> p t d", d=DHIP)
                    nc.gpsimd.dma_start(out=xhi3[:, :, 0:DHI], in_=x_d[bsl, tsl, DLO:D])
                    # zero the pad columns so the xbar transpose never reads
                    # uninitialized SBUF (transposed pad lands on partitions
                    # 72:128 of xThi, which no matmul reads)
                    nc.vector.memset(xhi3[:, :, DHI:DHIP], 0.0)
                    xTlo = xtpool.tile([128, tc_load * 128], f16, tag="xtlo")
                    xThi = xtpool.tile([DHIP, tc_load * 128], f16, tag="xthi")
                    nc.sync.dma_start(
                        out=xTlo[:].rearrange("p (t b) -> p t b", b=128),
                        in_=xlo_t[:],
                        transpose=True,
                    )
                    nc.sync.dma_start(
                        out=xThi[:].rearrange("p (t b) -> p t b", b=128),
                        in_=xhi_t[:],
                        transpose=True,
                    )
                    xt_tiles.append((xTlo, xThi))

                for k in range(nblk):
                    xTlo, xThi = xt_tiles[k]
                    for cp in range(tc_load // tc_ps):
                        off = cp * tc_ps * 128
                        ncols = tc_ps * 128
                        ps = pspool.tile([64, ncols], f32, tag=f"ps{k}")
                        for j in range(ncols // 512):
                            jsl = slice(j * 512, (j + 1) * 512)
                            xsl = slice(off + j * 512, off + (j + 1) * 512)
                            nc.tensor.matmul(
                                ps[:, jsl], wihT_lo16[:], xTlo[:, xsl],
                                start=True, stop=False, skip_group_check=True,
                            )
                            nc.tensor.matmul(
                                ps[:, jsl], wihT_hi16[0:DHI, :], xThi[0:DHI, xsl],
                                start=False, stop=False, skip_group_check=True,
                            )
                        for tt in range(tc_ps):
                            sl = ps[:, tt * 128 : (tt + 1) * 128]
                            # stop on the last matmul touching each 2KB PSUM
                            # zero-region (512 fp32 cols = 4 t-steps); the
                            # group checker can't model per-element
                            # has_written interleaving, so skip it
                            nc.tensor.matmul(
                                sl, whhT[:], hs[k][0:64, :],
                                start=False, stop=(tt % 4 == 3),
                                skip_group_check=True,
                            )
                            nc.scalar.activation(
                                hs[k][0:64, :], sl, Tanh, bias=bias[:, 0:1]
                            )

            # ---- output head ----
            for k in range(nblk):
                ps2 = pspool.tile([128, 1], f32, tag=f"ps{k}")
                nc.tensor.matmul(ps2[:], hs[k][:, :], waug[:], start=True, stop=True)
                ob = hpool.tile([128, 1], f32, tag=f"ob{k}")
                nc.scalar.activation(ob[:], ps2[:], Sigmoid)
                nc.scalar.dma_start(out=out_d[k * 128 : (k + 1) * 128, :], in_=ob[:])

    nc.compile()
    return nc


_CACHE = {}


def _get_program():
    if "nc" not in _CACHE:
        _CACHE["nc"] = build_program()
    return _CACHE["nc"]


def kernel(**inputs):
    from concourse.bass_utils import run_bass_kernel_spmd

    nc = _get_program()
    x = np.ascontiguousarray(inputs["x"])
    shared = {
        "W_ih": np.ascontiguousarray(inputs["W_ih"]),
        "W_hh": np.ascontiguousarray(inputs["W_hh"]),
        "b_ih": np.ascontiguousarray(inputs["b_ih"]),
        "b_hh": np.ascontiguousarray(inputs["b_hh"]),
        "W_out": np.ascontiguousarray(inputs["W_out"]),
        "b_out": np.ascontiguousarray(inputs["b_out"]),
    }
    in_maps = [
        dict(shared, x=np.ascontiguousarray(x[i * BS : (i + 1) * BS]))
        for i in range(N_CORES)
    ]
    res = run_bass_kernel_spmd(nc, in_maps, core_ids=list(range(N_CORES)))
    return np.concatenate([res.results[i]["out"] for i in range(N_CORES)], axis=0)


# revision 11
# speedup vs baseline: 1.3827x; 1.3827x over previous
"""RNN forward kernel for Trainium2, data-parallel over batch on 8 NeuronCores.

Computation (per batch row b):
    xp[t] = W_ih @ x[b,t] + b_ih + b_hh          (input projection, d=200 -> h=64)
    h_t   = tanh(xp[t] + W_hh @ h_{t-1})         (recurrence over T=128)
    out_b = sigmoid(W_out @ h_T + b_out)

Per-core plan (512 batch rows):
  - x streamed from HBM in (b-block, t-chunk) units, cast fp32->fp16 during the
    SWDGE DMA, staged in two layouts (d 0:128 and d 128:200 padded to 80) so the
    xbar DMA-transpose can run on large 2D-contiguous inputs.
  - xbar transpose puts d on partitions: xT[d, t, b] tiles feed the PE.
  - Projection matmuls in fp16 (PSUM accumulates fp32), recurrence matmul in
    fp32, tanh(+bias) on ScalarE straight out of PSUM.
  - Final: one matmul against [W_out.T; b_out] with a ones-row appended to h,
    sigmoid on ScalarE, store.
"""

import os

os.environ.setdefault("JAX_PLATFORMS", "cpu")

import numpy as np

B, T, D, H = 4096, 128, 200, 64
N_CORES = 8
BS = B // N_CORES  # 512 rows per core
DLO = 128          # first d-chunk (partitions of xT_lo)
DHI = D - DLO      # 72
# The hi d-chunk is padded to a full 128-column period: the HW xbar transpose
# only produces the plain 2D transpose when the source fold period equals the
# 128-column xbar tile width (verified on HW; 80 scrambles).
DHIP = 128
TC_LOAD = 16       # t granularity of load + transpose
TC_PS = 8          # t granularity of one PSUM accumulation chunk


def build_program(bs=BS, t_len=T, tc_load=TC_LOAD, tc_ps=TC_PS):
    import concourse.bacc as bacc
    import concourse.mybir as mybir
    import concourse.tile as tile
    from concourse._compat import axon_active

    f32, f16 = mybir.dt.float32, mybir.dt.float16
    Tanh = mybir.ActivationFunctionType.Tanh
    Sigmoid = mybir.ActivationFunctionType.Sigmoid

    nblk = bs // 128
    nld = t_len // tc_load

    nc = bacc.Bacc("TRN2", target_bir_lowering=False, debug=not axon_active())

    x_d = nc.dram_tensor("x", [bs, t_len, D], f32, kind="ExternalInput")
    wih_d = nc.dram_tensor("W_ih", [H, D], f32, kind="ExternalInput")
    whh_d = nc.dram_tensor("W_hh", [H, H], f32, kind="ExternalInput")
    bih_d = nc.dram_tensor("b_ih", [H], f32, kind="ExternalInput")
    bhh_d = nc.dram_tensor("b_hh", [H], f32, kind="ExternalInput")
    wout_d = nc.dram_tensor("W_out", [1, H], f32, kind="ExternalInput")
    bout_d = nc.dram_tensor("b_out", [1], f32, kind="ExternalInput")
    out_d = nc.dram_tensor("out", [bs, 1], f32, kind="ExternalOutput")

    with tile.TileContext(nc) as tc:
        with (
            tc.tile_pool(name="const", bufs=1) as cpool,
            tc.tile_pool(name="stage", bufs=3) as spool,
            tc.tile_pool(name="xt", bufs=8) as xtpool,
            tc.tile_pool(name="state", bufs=1) as hpool,
            tc.tile_pool(name="psum", bufs=1, space="PSUM") as pspool,
        ):
            # ---- weight prep (tiny, once) ----
            wih_nat = cpool.tile([64, 224], f32, tag="wihnat")
            nc.gpsimd.memset(wih_nat[:], 0.0)
            nc.scalar.dma_start(out=wih_nat[:, 0:D], in_=wih_d[:, :])

            wihT_lo = cpool.tile([128, 64], f32, tag="wihTlo")
            wihT_hi = cpool.tile([96, 64], f32, tag="wihThi")
            nc.gpsimd.memset(wihT_hi[:], 0.0)
            for bi in range(7):
                dst = wihT_lo if bi < 4 else wihT_hi
                r = (bi % 4) * 32 if bi < 4 else (bi - 4) * 32
                for bj in range(2):
                    nc.vector.transpose(
                        dst[r : r + 32, bj * 32 : (bj + 1) * 32],
                        wih_nat[bj * 32 : (bj + 1) * 32, bi * 32 : (bi + 1) * 32],
                    )
            wihT_lo16 = cpool.tile([128, 64], f16, tag="wihTlo16")
            wihT_hi16 = cpool.tile([96, 64], f16, tag="wihThi16")
            nc.vector.tensor_copy(wihT_lo16[:], wihT_lo[:])
            nc.vector.tensor_copy(wihT_hi16[:], wihT_hi[:])

            wh_nat = cpool.tile([64, 64], f32, tag="whnat")
            nc.scalar.dma_start(out=wh_nat[:], in_=whh_d[:, :])
            whhT = cpool.tile([64, 64], f32, tag="whhT")
            for bi in range(2):
                for bj in range(2):
                    nc.vector.transpose(
                        whhT[bi * 32 : (bi + 1) * 32, bj * 32 : (bj + 1) * 32],
                        wh_nat[bj * 32 : (bj + 1) * 32, bi * 32 : (bi + 1) * 32],
                    )
            whhT16 = cpool.tile([64, 64], f16, tag="whhT16")
            nc.vector.tensor_copy(whhT16[:], whhT[:])

            ba = cpool.tile([64, 1], f32, tag="ba")
            bb = cpool.tile([64, 1], f32, tag="bb")
            nc.scalar.dma_start(out=ba[:], in_=bih_d[:])
            nc.scalar.dma_start(out=bb[:], in_=bhh_d[:])
            bias = cpool.tile([64, 1], f32, tag="bias")
            nc.vector.tensor_add(bias[:], ba[:], bb[:])

            waug = cpool.tile([65, 1], f32, tag="waug")
            nc.scalar.dma_start(out=waug[0:64, :], in_=wout_d[0, :])
            nc.scalar.dma_start(out=waug[64:65, :], in_=bout_d[:])
            waug16 = cpool.tile([65, 1], f16, tag="waug16")
            nc.vector.tensor_copy(waug16[:], waug[:])

            # ---- recurrent state (ones row at partition 64 folds b_out in) ----
            hs = []
            for k in range(nblk):
                hk = hpool.tile([65, 128], f16, tag=f"h{k}")
                nc.gpsimd.memset(hk[0:64, :], 0.0)
                nc.gpsimd.memset(hk[64:65, :], 1.0)
                hs.append(hk)

            # ---- streaming pipeline ----
            for cl in range(nld):
                tsl = slice(cl * tc_load, (cl + 1) * tc_load)
                xt_tiles = []
                for k in range(nblk):
                    bsl = slice(k * 128, (k + 1) * 128)
                    # one fully-contiguous fp32 load (12.8KB per partition per
                    # descriptor); the fp16 cast + d-split happens on DVE so
                    # the DMA descriptors stay large
                    xstage = spool.tile([128, tc_load * D], f32, tag="xst")
                    nc.scalar.dma_start(out=xstage[:], in_=x_d[bsl, tsl, :])
                    xs3 = xstage[:].rearrange("p (t d) -> p t d", d=D)
                    xlo_t = spool.tile([128, tc_load * DLO], f16, tag="xlo")
                    xhi_t = spool.tile([128, tc_load * DHIP], f16, tag="xhi")
                    nc.vector.tensor_copy(
                        xlo_t[:].rearrange("p (t d) -> p t d", d=DLO),
                        xs3[:, :, 0:DLO],
                    )
                    xhi3 = xhi_t[:].rearrange("p (t d) -> p t d", d=DHIP)
                    nc.vector.tensor_copy(xhi3[:, :, 0:DHI], xs3[:, :, DLO:D])
                    # zero the pad columns so the xbar transpose never reads
                    # uninitialized SBUF (transposed pad lands on partitions
                    # 72:128 of xThi, which no matmul reads)
                    nc.vector.memset(xhi3[:, :, DHI:DHIP], 0.0)
                    xTlo = xtpool.tile([128, tc_load * 128], f16, tag="xtlo")
                    xThi = xtpool.tile([DHIP, tc_load * 128], f16, tag="xthi")
                    nc.sync.dma_start(
                        out=xTlo[:].rearrange("p (t b) -> p t b", b=128),
                        in_=xlo_t[:],
                        transpose=True,
                    )
                    nc.sync.dma_start(
                        out=xThi[:].rearrange("p (t b) -> p t b", b=128),
                        in_=xhi_t[:],
                        transpose=True,
                    )
                    xt_tiles.append((xTlo, xThi))

                for k in range(nblk):
                    xTlo, xThi = xt_tiles[k]
                    for cp in range(tc_load // tc_ps):
                        off = cp * tc_ps * 128
                        ncols = tc_ps * 128
                        ps = pspool.tile([64, ncols], f32, tag=f"ps{k}")
                        for j in range(ncols // 512):
                            jsl = slice(j * 512, (j + 1) * 512)
                            xsl = slice(off + j * 512, off + (j + 1) * 512)
                            nc.tensor.matmul(
                                ps[:, jsl], wihT_lo16[:], xTlo[:, xsl],
                                start=True, stop=False, skip_group_check=True,
                            )
                            nc.tensor.matmul(
                                ps[:, jsl], wihT_hi16[0:DHI, :], xThi[0:DHI, xsl],
                                start=False, stop=False, skip_group_check=True,
                            )
                        for tt in range(tc_ps):
                            sl = ps[:, tt * 128 : (tt + 1) * 128]
                            # stop on the last matmul touching each 2KB PSUM
                            # zero-region (512 fp32 cols = 4 t-steps); the
                            # group checker can't model per-element
                            # has_written interleaving, so skip it
                            nc.tensor.matmul(
                                sl, whhT16[:], hs[k][0:64, :],
                                start=False, stop=(tt % 4 == 3),
                                skip_group_check=True,
                            )
                            nc.scalar.activation(
                                hs[k][0:64, :], sl, Tanh, bias=bias[:, 0:1]
                            )

            # ---- output head ----
            for k in range(nblk):
                ps2 = pspool.tile([128, 1], f32, tag=f"ps{k}")
                nc.tensor.matmul(ps2[:], hs[k][:, :], waug16[:], start=True, stop=True)
                ob = hpool.tile([128, 1], f32, tag=f"ob{k}")
                nc.scalar.activation(ob[:], ps2[:], Sigmoid)
                nc.scalar.dma_start(out=out_d[k * 128 : (k + 1) * 128, :], in_=ob[:])

    nc.compile()
    return nc


_CACHE = {}


def _get_program():
    if "nc" not in _CACHE:
        _CACHE["nc"] = build_program()
    return _CACHE["nc"]


def kernel(**inputs):
    from concourse.bass_utils import run_bass_kernel_spmd

    nc = _get_program()
    x = np.ascontiguousarray(inputs["x"])
    shared = {
        "W_ih": np.ascontiguousarray(inputs["W_ih"]),
        "W_hh": np.ascontiguousarray(inputs["W_hh"]),
        "b_ih": np.ascontiguousarray(inputs["b_ih"]),
        "b_hh": np.ascontiguousarray(inputs["b_hh"]),
        "W_out": np.ascontiguousarray(inputs["W_out"]),
        "b_out": np.ascontiguousarray(inputs["b_out"]),
    }
    in_maps = [
        dict(shared, x=np.ascontiguousarray(x[i * BS : (i + 1) * BS]))
        for i in range(N_CORES)
    ]
    res = run_bass_kernel_spmd(nc, in_maps, core_ids=list(range(N_CORES)))
    return np.concatenate([res.results[i]["out"] for i in range(N_CORES)], axis=0)


# revision 14
# speedup vs baseline: 1.4242x; 1.0300x over previous
"""RNN forward kernel for Trainium2, data-parallel over batch on 8 NeuronCores.

Computation (per batch row b):
    xp[t] = W_ih @ x[b,t] + b_ih + b_hh          (input projection, d=200 -> h=64)
    h_t   = tanh(xp[t] + W_hh @ h_{t-1})         (recurrence over T=128)
    out_b = sigmoid(W_out @ h_T + b_out)

Per-core plan (512 batch rows):
  - x streamed from HBM in (b-block, t-chunk) units, cast fp32->fp16 during the
    SWDGE DMA, staged in two layouts (d 0:128 and d 128:200 padded to 80) so the
    xbar DMA-transpose can run on large 2D-contiguous inputs.
  - xbar transpose puts d on partitions: xT[d, t, b] tiles feed the PE.
  - Projection matmuls in fp16 (PSUM accumulates fp32), recurrence matmul in
    fp32, tanh(+bias) on ScalarE straight out of PSUM.
  - Final: one matmul against [W_out.T; b_out] with a ones-row appended to h,
    sigmoid on ScalarE, store.
"""

import os

os.environ.setdefault("JAX_PLATFORMS", "cpu")

import numpy as np

B, T, D, H = 4096, 128, 200, 64
N_CORES = 8
BS = B // N_CORES  # 512 rows per core
DLO = 128          # first d-chunk (partitions of xT_lo)
DHI = D - DLO      # 72
# The hi d-chunk is padded to a full 128-column period: the HW xbar transpose
# only produces the plain 2D transpose when the source fold period equals the
# 128-column xbar tile width (verified on HW; 80 scrambles).
DHIP = 128
TC_LOAD = 16       # t granularity of load + transpose
TC_PS = 8          # t granularity of one PSUM accumulation chunk


def build_program(bs=BS, t_len=T, tc_load=TC_LOAD, tc_ps=TC_PS):
    import concourse.bacc as bacc
    import concourse.mybir as mybir
    import concourse.tile as tile
    from concourse._compat import axon_active

    f32, f16 = mybir.dt.float32, mybir.dt.float16
    Tanh = mybir.ActivationFunctionType.Tanh
    Sigmoid = mybir.ActivationFunctionType.Sigmoid

    nblk = bs // 128
    nld = t_len // tc_load

    nc = bacc.Bacc("TRN2", target_bir_lowering=False, debug=not axon_active())

    x_d = nc.dram_tensor("x", [bs, t_len, D], f32, kind="ExternalInput")
    wih_d = nc.dram_tensor("W_ih", [H, D], f32, kind="ExternalInput")
    whh_d = nc.dram_tensor("W_hh", [H, H], f32, kind="ExternalInput")
    bih_d = nc.dram_tensor("b_ih", [H], f32, kind="ExternalInput")
    bhh_d = nc.dram_tensor("b_hh", [H], f32, kind="ExternalInput")
    wout_d = nc.dram_tensor("W_out", [1, H], f32, kind="ExternalInput")
    bout_d = nc.dram_tensor("b_out", [1], f32, kind="ExternalInput")
    out_d = nc.dram_tensor("out", [bs, 1], f32, kind="ExternalOutput")

    with tile.TileContext(nc) as tc:
        with (
            tc.tile_pool(name="const", bufs=1) as cpool,
            tc.tile_pool(name="stage", bufs=3) as spool,
            tc.tile_pool(name="xt", bufs=10) as xtpool,
            tc.tile_pool(name="state", bufs=1) as hpool,
            tc.tile_pool(name="psum", bufs=1, space="PSUM") as pspool,
        ):
            # ---- weight prep (tiny, once) ----
            wih_nat = cpool.tile([64, 224], f32, tag="wihnat")
            nc.gpsimd.memset(wih_nat[:], 0.0)
            nc.scalar.dma_start(out=wih_nat[:, 0:D], in_=wih_d[:, :])

            wihT_lo = cpool.tile([128, 64], f32, tag="wihTlo")
            wihT_hi = cpool.tile([96, 64], f32, tag="wihThi")
            nc.gpsimd.memset(wihT_hi[:], 0.0)
            for bi in range(7):
                dst = wihT_lo if bi < 4 else wihT_hi
                r = (bi % 4) * 32 if bi < 4 else (bi - 4) * 32
                for bj in range(2):
                    nc.vector.transpose(
                        dst[r : r + 32, bj * 32 : (bj + 1) * 32],
                        wih_nat[bj * 32 : (bj + 1) * 32, bi * 32 : (bi + 1) * 32],
                    )
            wihT_lo16 = cpool.tile([128, 64], f16, tag="wihTlo16")
            wihT_hi16 = cpool.tile([96, 64], f16, tag="wihThi16")
            nc.vector.tensor_copy(wihT_lo16[:], wihT_lo[:])
            nc.vector.tensor_copy(wihT_hi16[:], wihT_hi[:])

            wh_nat = cpool.tile([64, 64], f32, tag="whnat")
            nc.scalar.dma_start(out=wh_nat[:], in_=whh_d[:, :])
            whhT = cpool.tile([64, 64], f32, tag="whhT")
            for bi in range(2):
                for bj in range(2):
                    nc.vector.transpose(
                        whhT[bi * 32 : (bi + 1) * 32, bj * 32 : (bj + 1) * 32],
                        wh_nat[bj * 32 : (bj + 1) * 32, bi * 32 : (bi + 1) * 32],
                    )
            whhT16 = cpool.tile([64, 64], f16, tag="whhT16")
            nc.vector.tensor_copy(whhT16[:], whhT[:])

            ba = cpool.tile([64, 1], f32, tag="ba")
            bb = cpool.tile([64, 1], f32, tag="bb")
            nc.scalar.dma_start(out=ba[:], in_=bih_d[:])
            nc.scalar.dma_start(out=bb[:], in_=bhh_d[:])
            bias = cpool.tile([64, 1], f32, tag="bias")
            nc.vector.tensor_add(bias[:], ba[:], bb[:])

            waug = cpool.tile([65, 1], f32, tag="waug")
            nc.scalar.dma_start(out=waug[0:64, :], in_=wout_d[0, :])
            nc.scalar.dma_start(out=waug[64:65, :], in_=bout_d[:])
            waug16 = cpool.tile([65, 1], f16, tag="waug16")
            nc.vector.tensor_copy(waug16[:], waug[:])

            # ---- recurrent state (ones row at partition 64 folds b_out in) ----
            hs = []
            for k in range(nblk):
                hk = hpool.tile([65, 128], f16, tag=f"h{k}")
                nc.gpsimd.memset(hk[0:64, :], 0.0)
                nc.gpsimd.memset(hk[64:65, :], 1.0)
                hs.append(hk)

            # ---- streaming pipeline ----
            for cl in range(nld):
                tsl = slice(cl * tc_load, (cl + 1) * tc_load)
                xt_tiles = []
                for k in range(nblk):
                    bsl = slice(k * 128, (k + 1) * 128)
                    # one fully-contiguous fp32 load (12.8KB per partition per
                    # descriptor); the fp16 cast + d-split happens on DVE so
                    # the DMA descriptors stay large
                    xstage = spool.tile([128, tc_load * D], f32, tag="xst")
                    nc.gpsimd.dma_start(out=xstage[:], in_=x_d[bsl, tsl, :])
                    xs3 = xstage[:].rearrange("p (t d) -> p t d", d=D)
                    xlo_t = spool.tile([128, tc_load * DLO], f16, tag="xlo")
                    xhi_t = spool.tile([128, tc_load * DHIP], f16, tag="xhi")
                    nc.vector.tensor_copy(
                        xlo_t[:].rearrange("p (t d) -> p t d", d=DLO),
                        xs3[:, :, 0:DLO],
                    )
                    xhi3 = xhi_t[:].rearrange("p (t d) -> p t d", d=DHIP)
                    nc.vector.tensor_copy(xhi3[:, :, 0:DHI], xs3[:, :, DLO:D])
                    # zero the pad columns so the xbar transpose never reads
                    # uninitialized SBUF (transposed pad lands on partitions
                    # 72:128 of xThi, which no matmul reads); u32 view halves
                    # the DVE element count
                    nc.vector.memset(
                        xhi3[:, :, DHI:DHIP].bitcast(mybir.dt.uint32), 0
                    )
                    xTlo = xtpool.tile([128, tc_load * 128], f16, tag="xtlo")
                    xThi = xtpool.tile([DHIP, tc_load * 128], f16, tag="xthi")
                    nc.sync.dma_start(
                        out=xTlo[:].rearrange("p (t b) -> p t b", b=128),
                        in_=xlo_t[:],
                        transpose=True,
                    )
                    nc.sync.dma_start(
                        out=xThi[:].rearrange("p (t b) -> p t b", b=128),
                        in_=xhi_t[:],
                        transpose=True,
                    )
                    xt_tiles.append((xTlo, xThi))

                for k in range(nblk):
                    xTlo, xThi = xt_tiles[k]
                    for cp in range(tc_load // tc_ps):
                        off = cp * tc_ps * 128
                        ncols = tc_ps * 128
                        ps = pspool.tile([64, ncols], f32, tag=f"ps{k}")
                        for j in range(ncols // 512):
                            jsl = slice(j * 512, (j + 1) * 512)
                            xsl = slice(off + j * 512, off + (j + 1) * 512)
                            nc.tensor.matmul(
                                ps[:, jsl], wihT_lo16[:], xTlo[:, xsl],
                                start=True, stop=False, skip_group_check=True,
                            )
                            nc.tensor.matmul(
                                ps[:, jsl], wihT_hi16[0:DHI, :], xThi[0:DHI, xsl],
                                start=False, stop=False, skip_group_check=True,
                            )
                        for tt in range(tc_ps):
                            sl = ps[:, tt * 128 : (tt + 1) * 128]
                            # stop on the last matmul touching each 2KB PSUM
                            # zero-region (512 fp32 cols = 4 t-steps); the
                            # group checker can't model per-element
                            # has_written interleaving, so skip it
                            nc.tensor.matmul(
                                sl, whhT16[:], hs[k][0:64, :],
                                start=False, stop=(tt % 4 == 3),
                                skip_group_check=True,
                            )
                            nc.scalar.activation(
                                hs[k][0:64, :], sl, Tanh, bias=bias[:, 0:1]
                            )

            # ---- output head ----
            for k in range(nblk):
                ps2 = pspool.tile([128, 1], f32, tag=f"ps{k}")
                nc.tensor.matmul(ps2[:], hs[k][:, :], waug16[:], start=True, stop=True)
                ob = hpool.tile([128, 1], f32, tag=f"ob{k}")
                nc.scalar.activation(ob[:], ps2[:], Sigmoid)
                nc.scalar.dma_start(out=out_d[k * 128 : (k + 1) * 128, :], in_=ob[:])

    nc.compile()
    return nc


_CACHE = {}


def _get_program():
    if "nc" not in _CACHE:
        _CACHE["nc"] = build_program()
    return _CACHE["nc"]


def kernel(**inputs):
    from concourse.bass_utils import run_bass_kernel_spmd

    nc = _get_program()
    x = np.ascontiguousarray(inputs["x"])
    shared = {
        "W_ih": np.ascontiguousarray(inputs["W_ih"]),
        "W_hh": np.ascontiguousarray(inputs["W_hh"]),
        "b_ih": np.ascontiguousarray(inputs["b_ih"]),
        "b_hh": np.ascontiguousarray(inputs["b_hh"]),
        "W_out": np.ascontiguousarray(inputs["W_out"]),
        "b_out": np.ascontiguousarray(inputs["b_out"]),
    }
    in_maps = [
        dict(shared, x=np.ascontiguousarray(x[i * BS : (i + 1) * BS]))
        for i in range(N_CORES)
    ]
    res = run_bass_kernel_spmd(nc, in_maps, core_ids=list(range(N_CORES)))
    return np.concatenate([res.results[i]["out"] for i in range(N_CORES)], axis=0)
